# revision 13
# baseline (speedup 1.0000x reference)
"""ChebNet (K=3, 2 layers) forward on 8 Trainium2 NeuronCores.

Math (edge weights fold into node scalings since norm[e] = -dis[src]*dis[dst]):
    P(h)   = -dis * (A_dst^T (dis * h))        A = 0/1 adjacency
    layer  = x@W0 + P(x@W1) + (2 P(P(x@W2)) - x@W2) + b
    out    = log_softmax(layer2(relu(layer1(x))))

Sharding: nodes dst-partitioned across 8 cores. Each sparse propagation:
  bf16 table of scaled features (AllGather'd), bulk indexed gather
  (dma_gather, int16 idx => 32768-row windows), segment-sum via one-hot
  fp8 [128 edge x 128 node] matmuls accumulated in PSUM per (window, block),
  accumulated across windows in an SBUF accumulator.

All FP math on device; host does integer preprocessing only (CSR sort,
padding, one-hot construction, layout permutes).
"""
import os
import numpy as np
import ml_dtypes

import concourse.bass as bass
import concourse.bacc as bacc
import concourse.mybir as mybir
import concourse.tile as tile
from concourse import library_config
from concourse._compat import cdiv

F32 = mybir.dt.float32
BF16 = mybir.dt.bfloat16
FP8 = mybir.dt.float8e4
I16 = mybir.dt.int16
AF = mybir.ActivationFunctionType
OP = mybir.AluOpType

P = 128
NCORES = 8


class CFG:
    N = 100000
    F1 = 128          # input features
    H = 64            # hidden
    C = 40            # classes
    WIN = 32768       # gather window rows (int16 idx)
    CH = 8            # gather chunk size in tiles

    @property
    def OWN(self):
        return self.N // NCORES

    @property
    def NB(self):
        return cdiv(self.OWN, P)

    @property
    def PADN(self):
        return self.NB * P

    @property
    def GPAD(self):
        return NCORES * self.PADN

    @property
    def NWIN(self):
        return cdiv(self.GPAD, self.WIN)


cfg = CFG()

_cache = {}


# ---------------------------------------------------------------- host prep
def _row_of(v, c):
    """Permuted table row of global node v owned by core c (partition-major)."""
    l = v - c * cfg.OWN
    return c * cfg.PADN + (l % P) * cfg.NB + (l // P)


def preprocess(x, edge_index, W1, b1, W2, b2):
    src = np.asarray(edge_index[0], dtype=np.int64)
    dst = np.asarray(edge_index[1], dtype=np.int64)
    deg = np.bincount(dst, minlength=cfg.N).astype(np.float32)

    owner_s = src // cfg.OWN
    grow_s = _row_of(src, owner_s)          # permuted global table row of src

    per_core = []
    for c in range(NCORES):
        m = (dst >= c * cfg.OWN) & (dst < (c + 1) * cfg.OWN)
        es = grow_s[m]
        ld = dst[m] - c * cfg.OWN
        b = ld // P
        pcol = ld % P
        w = es // cfg.WIN
        order = np.lexsort((ld, b, w))
        per_core.append((w[order], b[order], es[order] % cfg.WIN, pcol[order]))

    # static per-(w,b) tile counts = max over cores
    t_wb = np.zeros((cfg.NWIN, cfg.NB), np.int64)
    for c in range(NCORES):
        w, b, _, _ = per_core[c]
        cnt = np.bincount(w * cfg.NB + b, minlength=cfg.NWIN * cfg.NB)
        t_wb = np.maximum(t_wb, cdiv_np(cnt.reshape(cfg.NWIN, cfg.NB), P))
    t_wb[0] = np.maximum(t_wb[0], 1)
    T = int(t_wb.sum())
    slots = T * P

    # group start slot offsets
    gstart = np.zeros(cfg.NWIN * cfg.NB + 1, np.int64)
    gstart[1:] = np.cumsum(t_wb.reshape(-1) * P)

    x = np.asarray(x, np.float32)
    core_inputs = []
    for c in range(NCORES):
        w, b, es, pcol = per_core[c]
        g = w * cfg.NB + b
        # slot of each edge: group base + rank within group
        changes = np.ones(len(g), bool)
        changes[1:] = g[1:] != g[:-1]
        grp_first = np.flatnonzero(changes)
        rank = np.arange(len(g)) - np.repeat(grp_first, np.diff(np.append(grp_first, len(g))))
        slot = gstart[g] + rank

        idx_full = np.zeros(slots, np.int64)           # window-local src row
        idx_full[slot] = es
        mcol = np.full(slots, -1, np.int64)            # one-hot col (-1 = pad)
        mcol[slot] = pcol

        # idx wrap: slot i -> [16g + i%16, i//16], int16
        iw = np.zeros((P, slots // 16), np.int16)
        a = idx_full.astype(np.int16)
        ii = np.arange(slots)
        for gg in range(8):
            iw[16 * gg + (ii % 16), ii // 16] = a

        # one-hot M [e, t, n] -> device layout [p=e, t, n] fp8
        mm = np.zeros((P, T, P), np.uint8)             # fp8e4m3 bits
        one = np.float32(1.0).astype(ml_dtypes.float8_e4m3).view(np.uint8)
        tt = ii // P
        ee = ii % P
        valid = mcol >= 0
        mm[ee[valid], tt[valid], mcol[valid]] = one
        mm = mm.view(ml_dtypes.float8_e4m3)

        # x slice permuted [p, b, f]
        xs = np.zeros((P, cfg.NB, cfg.F1), np.float32)
        n = np.arange(cfg.OWN)
        xs[n % P, n // P, :] = x[c * cfg.OWN:(c + 1) * cfg.OWN]
        dg = np.zeros((P, cfg.NB), np.float32)
        dg[n % P, n // P] = deg[c * cfg.OWN:(c + 1) * cfg.OWN]

        core_inputs.append(dict(
            x_in=xs, deg_in=dg, idx_in=iw, m_in=np.ascontiguousarray(mm),
        ))

    W1 = np.asarray(W1, np.float32)
    W2 = np.asarray(W2, np.float32)
    w1cat = np.concatenate([W1[1], W1[2], W1[0]], axis=1)       # [F1, 3H]
    w2cat = np.concatenate([W2[1], W2[2], W2[0]], axis=1)       # [H, 3C]
    shared = dict(
        w1_in=np.ascontiguousarray(w1cat),
        w2_in=np.ascontiguousarray(w2cat),
        b1_in=np.ascontiguousarray(np.broadcast_to(np.asarray(b1, np.float32), (P, cfg.H)).copy()),
        b2_in=np.ascontiguousarray(np.broadcast_to(np.asarray(b2, np.float32), (P, cfg.C)).copy()),
        ident_in=np.eye(P, dtype=np.float32),
    )
    for ci in core_inputs:
        ci.update(shared)
    return core_inputs, t_wb


def cdiv_np(a, b):
    return -(-a // b)


# ---------------------------------------------------------------- device build
def build_bass(t_wb, max_phase=99):
    NB, NWIN, F1, H, C = cfg.NB, cfg.NWIN, cfg.F1, cfg.H, cfg.C
    T = int(t_wb.sum())
    TABROWS = NWIN * cfg.WIN

    nc = bacc.Bacc("TRN2", target_bir_lowering=False, debug=False,
                   num_devices=NCORES, num_swdge_queues=4)

    x_in = nc.dram_tensor("x_in", [P, NB, F1], F32, kind="ExternalInput")
    deg_in = nc.dram_tensor("deg_in", [P, NB], F32, kind="ExternalInput")
    idx_in = nc.dram_tensor("idx_in", [P, T * P // 16], I16, kind="ExternalInput")
    m_in = nc.dram_tensor("m_in", [P, T, P], FP8, kind="ExternalInput")
    w1_in = nc.dram_tensor("w1_in", [F1, 3 * H], F32, kind="ExternalInput")
    w2_in = nc.dram_tensor("w2_in", [H, 3 * C], F32, kind="ExternalInput")
    b1_in = nc.dram_tensor("b1_in", [P, H], F32, kind="ExternalInput")
    b2_in = nc.dram_tensor("b2_in", [P, C], F32, kind="ExternalInput")
    ident_in = nc.dram_tensor("ident_in", [P, P], F32, kind="ExternalInput")
    out_d = nc.dram_tensor("out_d", [P, NB * C], F32, kind="ExternalOutput")

    ag_in = [nc.dram_tensor(f"ag{i}_in", [cfg.PADN, 2 * H], BF16, kind="Internal")
             for i in range(4)]
    tabs = [nc.dram_tensor(f"tab{i}", [cfg.GPAD, 2 * H], BF16, kind="Internal",
                           addr_space="Shared")
            for i in range(4)]

    CH = cfg.CH
    rg = [list(range(NCORES))]

    with tile.TileContext(nc) as tc:
        import contextlib
        ctx = contextlib.ExitStack()
        with ctx:
            cp = ctx.enter_context(tc.tile_pool(name="consts", bufs=1))
            bigp = ctx.enter_context(tc.tile_pool(name="big", bufs=1))
            xp = ctx.enter_context(tc.tile_pool(name="xin", bufs=3))
            gp = ctx.enter_context(tc.tile_pool(name="gather", bufs=3))
            mp = ctx.enter_context(tc.tile_pool(name="mtiles", bufs=3))
            ip = ctx.enter_context(tc.tile_pool(name="idxs", bufs=3))
            fp = ctx.enter_context(tc.tile_pool(name="fin", bufs=4))
            pp = ctx.enter_context(tc.tile_pool(name="psum", bufs=6, space="PSUM"))
            pfin = pp

            nc.gpsimd.load_library(library_config.mlp)

            # ---------- constants ----------
            wcat1 = cp.tile([F1, 3 * H], F32)
            nc.sync.dma_start(out=wcat1[:], in_=w1_in[:])
            # cols [2H:3H] := W1[0] - W1[2]
            nc.vector.tensor_tensor(out=wcat1[:, 2 * H:3 * H], in0=wcat1[:, 2 * H:3 * H],
                                    in1=wcat1[:, H:2 * H], op=OP.subtract)
            wcat2 = cp.tile([H, 4 * C], F32)
            nc.sync.dma_start(out=wcat2[:, :3 * C], in_=w2_in[:])
            nc.vector.tensor_tensor(out=wcat2[:, 3 * C:], in0=wcat2[:, 2 * C:3 * C],
                                    in1=wcat2[:, C:2 * C], op=OP.subtract)
            b1r = cp.tile([P, H], F32)
            nc.sync.dma_start(out=b1r[:], in_=b1_in[:])
            b2r = cp.tile([P, C], F32)
            nc.sync.dma_start(out=b2r[:], in_=b2_in[:])
            ident = cp.tile([P, P], F32)
            nc.sync.dma_start(out=ident[:], in_=ident_in[:])

            degt = cp.tile([P, NB], F32)
            nc.sync.dma_start(out=degt[:], in_=deg_in[:])
            # dis = (deg>0) * sqrt(1/max(deg,1))
            dis = cp.tile([P, NB], F32)
            tmpc = cp.tile([P, NB], F32)
            nc.vector.tensor_scalar_max(tmpc[:], degt[:], 1.0)
            nc.vector.reciprocal(tmpc[:], tmpc[:])
            nc.scalar.activation(tmpc[:], tmpc[:], AF.Sqrt)
            mask = cp.tile([P, NB], F32)
            nc.vector.tensor_scalar(mask[:], degt[:], 0.0, None, op0=OP.is_gt)
            nc.vector.tensor_tensor(out=dis[:], in0=tmpc[:], in1=mask[:], op=OP.mult)
            negdis = cp.tile([P, NB], F32)
            nc.vector.tensor_scalar_mul(negdis[:], dis[:], -1.0)
            negdis2 = cp.tile([P, NB], F32)   # -dis^2
            nc.vector.tensor_tensor(out=negdis2[:], in0=negdis[:], in1=dis[:], op=OP.mult)
            neg2dis = cp.tile([P, NB], F32)   # -2 dis
            nc.vector.tensor_scalar_mul(neg2dis[:], dis[:], -2.0)

            # ---------- big SBUF state ----------
            y0my2 = bigp.tile([P, NB * H], F32)
            z1 = bigp.tile([P, NB * H], F32)
            acc = bigp.tile([P, NB * F1], F32)
            tabsb = bigp.tile([P, NB, 2 * H], BF16)
            out2 = bigp.tile([P, NB * C], F32)

            # ---------- phase A: y = x @ W1cat; write ytilde table ----------
            def scope(name):
                sid, _ = nc.enter_named_scope(name, False)
                return lambda: nc.leave_named_scope(name, sid, False)

            if max_phase >= 1:
              _s = scope("densA")
              for b in range(NB):
                  xs = xp.tile([P, F1], F32, tag="xs")
                  nc.sync.dma_start(out=xs[:], in_=x_in[:, b, :])
                  xt_ps = pp.tile([P, F1], F32, tag="ps")
                  nc.tensor.transpose(out=xt_ps[:], in_=xs[:], identity=ident[:])
                  xt = xp.tile([P, F1], F32, tag="xt")
                  nc.vector.tensor_copy(out=xt[:], in_=xt_ps[:])
                  y_ps = pp.tile([P, 3 * H], F32, tag="ps")
                  nc.tensor.matmul(y_ps[:], lhsT=xt[:], rhs=wcat1[:], start=True, stop=True)
                  # ytilde (bf16) = dis * y[:, :2H]
                  nc.vector.tensor_scalar(tabsb[:, b, :], y_ps[:, :2 * H], dis[:, b:b + 1],
                                          None, op0=OP.mult)
                  nc.scalar.activation(y0my2[:, b * H:(b + 1) * H], y_ps[:, 2 * H:3 * H], AF.Copy)
              _s()
              _s = scope("ag0")
              nc.sync.dma_start(out=ag_in[0][:], in_=tabsb[:])
              nc.gpsimd.collective_compute(
                  "AllGather", OP.bypass, replica_groups=rg,
                  ins=[ag_in[0][:].opt()], outs=[tabs[0][:].opt()])
              _s()

              # ---------- sparse propagation pass ----------
              def prop_pass(pi, Fi):
                  tab = tabs[pi]
                  gtile = None
                  gt0 = 0                      # first global tile of this chunk
                  nloc = 0
                  for w in range(NWIN):
                      tw = int(t_wb[w].sum())
                      done = 0
                      while done < tw:
                          ch = min(CH, tw - done)
                          idx_t = ip.tile([P, CH * 8], I16, tag="idx")
                          nc.sync.dma_start(out=idx_t[:, :ch * 8],
                                            in_=idx_in[:, gt0 * 8:(gt0 + ch) * 8])
                          gb = gp.tile([P, CH, Fi], BF16, tag=f"gb{Fi}")
                          wend = min((w + 1) * cfg.WIN, cfg.GPAD)
                          nc.gpsimd.dma_gather(
                              gb[:, :ch, :], tab[w * cfg.WIN:wend, :],
                              idx_t[:, :ch * 8], ch * P, ch * P, Fi)
                          mt = mp.tile([P, CH, P], FP8, tag="mt")
                          nc.sync.dma_start(out=mt[:, :ch, :], in_=m_in[:, gt0:gt0 + ch, :])
                          yield (w, gt0, ch, gb, mt)
                          gt0 += ch
                          done += ch

              def run_pass(pi, Fi):
                  # bookkeeping over groups (w, b)
                  starts = {}
                  for wv in range(NWIN):
                      base = int(t_wb[:wv].sum()) if wv else 0
                      off = 0
                      for b in range(NB):
                          starts[(wv, b)] = base + off
                          off += int(t_wb[wv, b])
                  # map global tile -> (w, b, first?, last?)
                  tmap = []
                  for wv in range(NWIN):
                      for b in range(NB):
                          n = int(t_wb[wv, b])
                          for k in range(n):
                              tmap.append((wv, b, k == 0, k == n - 1))
                  ps_cur = [None]

                  for (wv, gt0, ch, gb, mt) in prop_pass(pi, Fi):
                      for k in range(ch):
                          gt = gt0 + k
                          w_, b_, first, last = tmap[gt]
                          if first:
                              ps_cur[0] = pp.tile([P, Fi], F32, tag="ps", name="ps_prop")
                          nc.tensor.matmul(ps_cur[0][:], lhsT=mt[:, k, :], rhs=gb[:, k, :],
                                           start=first, stop=last)
                          if last:
                              dstsl = acc[:, b_ * F1:b_ * F1 + Fi]
                              if w_ == 0:
                                  nc.vector.tensor_copy(out=dstsl, in_=ps_cur[0][:])
                              else:
                                  nc.vector.tensor_tensor(out=dstsl, in0=dstsl,
                                                          in1=ps_cur[0][:], op=OP.add)

            # ===== pass 1 (F=2H on ytilde table) =====
            if max_phase >= 2:
              _s = scope("prop1")
              for _rep in range(int(os.environ.get("KERNEL_REPS", "1"))):
                run_pass(0, 2 * H)
              _s()
              _s = scope("post1")
              for b in range(NB):
                  a0 = acc[:, b * F1:b * F1 + H]
                  a1 = acc[:, b * F1 + H:b * F1 + 2 * H]
                  nc.vector.tensor_scalar(z1[:, b * H:(b + 1) * H], a0, negdis[:, b:b + 1],
                                          None, op0=OP.mult)
                  nc.vector.tensor_scalar(tabsb[:, b, :H], a1, negdis2[:, b:b + 1],
                                          None, op0=OP.mult)
              _s()
              _s = scope("ag1")
              nc.sync.dma_start(out=ag_in[1][:], in_=tabsb[:])
              nc.gpsimd.collective_compute(
                  "AllGather", OP.bypass, replica_groups=rg,
                  ins=[ag_in[1][:].opt()], outs=[tabs[1][:].opt()])
              _s()

            # ===== pass 2: acc = sum ztilde2[src]; h = relu(...) =====
            if max_phase >= 3:
              _s = scope("prop2")
              run_pass(1, 2 * H)
              _s()
              _s = scope("post2")
              for b in range(NB):
                  av = acc[:, b * F1:b * F1 + H]
                  h = fp.tile([P, H], F32, tag="h")
                  # h = relu(y0my2 + z1 + neg2dis*acc + b1)
                  nc.vector.tensor_scalar(h[:], av, neg2dis[:, b:b + 1], None, op0=OP.mult)
                  nc.vector.tensor_tensor(out=h[:], in0=h[:], in1=z1[:, b * H:(b + 1) * H], op=OP.add)
                  nc.vector.tensor_tensor(out=h[:], in0=h[:], in1=y0my2[:, b * H:(b + 1) * H], op=OP.add)
                  nc.vector.tensor_tensor(out=h[:], in0=h[:], in1=b1r[:], op=OP.add)
                  nc.vector.tensor_scalar_max(h[:], h[:], 0.0)
                  nc.vector.tensor_scalar(tabsb[:, b, :H], h[:], dis[:, b:b + 1],
                                          None, op0=OP.mult)
                  ht_ps = pfin.tile([P, P], F32, tag="ps")
                  nc.tensor.transpose(out=ht_ps[:H, :], in_=h[:], identity=ident[:])
                  ht = fp.tile([H, P], F32, tag="ht")
                  nc.vector.tensor_copy(out=ht[:], in_=ht_ps[:H, :])
                  lg = pfin.tile([P, C], F32, tag="ps")
                  nc.tensor.matmul(lg[:], lhsT=ht[:], rhs=wcat2[:, 3 * C:], start=True, stop=True)
                  nc.scalar.activation(out2[:, b * C:(b + 1) * C], lg[:], AF.Copy)
              _s()
              _s = scope("ag2")
              nc.sync.dma_start(out=ag_in[2][:], in_=tabsb[:])
              nc.gpsimd.collective_compute(
                  "AllGather", OP.bypass, replica_groups=rg,
                  ins=[ag_in[2][:].opt()], outs=[tabs[2][:].opt()])
              _s()

            # ===== pass 3: acc = sum htilde[src]; Q1 =====
            if max_phase >= 4:
              _s = scope("prop3")
              run_pass(2, 2 * H)
              _s()
              _s = scope("post3")
              for b in range(NB):
                  av = acc[:, b * F1:b * F1 + H]
                  q1 = fp.tile([P, H], F32, tag="q1")
                  nc.vector.tensor_scalar(q1[:], av, negdis[:, b:b + 1], None, op0=OP.mult)
                  nc.vector.tensor_scalar(tabsb[:, b, :H], av, negdis2[:, b:b + 1],
                                          None, op0=OP.mult)
                  qt_ps = pfin.tile([P, P], F32, tag="ps")
                  nc.tensor.transpose(out=qt_ps[:H, :], in_=q1[:], identity=ident[:])
                  qt = fp.tile([H, P], F32, tag="ht")
                  nc.vector.tensor_copy(out=qt[:], in_=qt_ps[:H, :])
                  lg = pfin.tile([P, C], F32, tag="ps")
                  nc.tensor.matmul(lg[:], lhsT=qt[:], rhs=wcat2[:, :C], start=True, stop=True)
                  o2 = out2[:, b * C:(b + 1) * C]
                  nc.vector.tensor_tensor(out=o2, in0=o2, in1=lg[:], op=OP.add)
              _s()
              _s = scope("ag3")
              nc.sync.dma_start(out=ag_in[3][:], in_=tabsb[:])
              nc.gpsimd.collective_compute(
                  "AllGather", OP.bypass, replica_groups=rg,
                  ins=[ag_in[3][:].opt()], outs=[tabs[3][:].opt()])
              _s()

            # ===== pass 4: acc = sum q1tilde[src]; logits + log_softmax =====
            if max_phase >= 5:
              _s = scope("prop4")
              run_pass(3, 2 * H)
              _s()
              _s = scope("post4")
              outsb = bigp.tile([P, NB * C], F32)
              for b in range(NB):
                  av = acc[:, b * F1:b * F1 + H]
                  q2 = fp.tile([P, H], F32, tag="q1")
                  nc.vector.tensor_scalar(q2[:], av, neg2dis[:, b:b + 1], None, op0=OP.mult)
                  qt_ps = pfin.tile([P, P], F32, tag="ps")
                  nc.tensor.transpose(out=qt_ps[:H, :], in_=q2[:], identity=ident[:])
                  qt = fp.tile([H, P], F32, tag="ht")
                  nc.vector.tensor_copy(out=qt[:], in_=qt_ps[:H, :])
                  lg = pfin.tile([P, C], F32, tag="ps")
                  nc.tensor.matmul(lg[:], lhsT=qt[:], rhs=wcat2[:, C:2 * C], start=True, stop=True)
                  lo = fp.tile([P, C], F32, tag="lo")
                  nc.vector.tensor_tensor(out=lo[:], in0=out2[:, b * C:(b + 1) * C],
                                          in1=lg[:], op=OP.add)
                  nc.vector.tensor_tensor(out=lo[:], in0=lo[:], in1=b2r[:], op=OP.add)
                  negmx = fp.tile([P, 1], F32, tag="negmx")
                  nc.vector.tensor_reduce(out=negmx[:], in_=lo[:], op=OP.max,
                                          axis=mybir.AxisListType.X, negate=True)
                  ex = fp.tile([P, C], F32, tag="ex")
                  sm = fp.tile([P, 1], F32, tag="sm")
                  nc.scalar.activation(ex[:], lo[:], AF.Exp, bias=negmx[:, :1],
                                       accum_out=sm[:, :1])
                  lns = fp.tile([P, 1], F32, tag="lns")
                  nc.scalar.activation(lns[:], sm[:], AF.Ln)
                  nc.vector.tensor_tensor(out=lns[:], in0=negmx[:], in1=lns[:],
                                          op=OP.subtract)
                  nc.vector.tensor_scalar(outsb[:, b * C:(b + 1) * C], lo[:],
                                          lns[:, :1], None, op0=OP.add)
              nc.sync.dma_start(out=out_d[:], in_=outsb[:])
              _s()

    nc.compile()
    return nc


# ---------------------------------------------------------------- entry point
def kernel(**inputs):
    core_inputs, t_wb = preprocess(
        inputs["x"], inputs["edge_index"], inputs["W1"], inputs["b1"],
        inputs["W2"], inputs["b2"])

    key = (cfg.N, tuple(t_wb.reshape(-1).tolist()))
    if key not in _cache:
        _cache[key] = build_bass(t_wb, int(os.environ.get("KERNEL_PHASES", "99")))
    nc = _cache[key]

    if os.environ.get("KERNEL_SIM"):
        from concourse.bass_interp import MultiCoreSim
        sim = MultiCoreSim(nc, NCORES)
        for c in range(NCORES):
            for k, v in core_inputs[c].items():
                sim.cores[c].tensor(k)[:] = v
        sim.simulate()
        results = [{"out_d": np.asarray(sim.cores[c].mem_tensor("out_d"))}
                   for c in range(NCORES)]
    else:
        from concourse.bass_utils import run_bass_kernel_spmd
        r = run_bass_kernel_spmd(nc, core_inputs, core_ids=list(range(NCORES)))
        results = r.results

    out = np.empty((cfg.N, cfg.C), np.float32)
    n = np.arange(cfg.OWN)
    for c in range(NCORES):
        od = results[c]["out_d"].reshape(P, cfg.NB, cfg.C)
        out[c * cfg.OWN:(c + 1) * cfg.OWN] = od[n % P, n // P, :]
    return out



# revision 24
# speedup vs baseline: 12.1153x; 12.1153x over previous
"""ChebNet (K=3, 2 layers) forward on 8 Trainium2 NeuronCores.

Math (edge weights fold into node scalings since norm[e] = -dis[src]*dis[dst]):
    P(h)   = -dis * (A_dst^T (dis * h))        A = 0/1 adjacency
    layer  = x@W0 + P(x@W1) + (2 P(P(x@W2)) - x@W2) + b
    out    = log_softmax(layer2(relu(layer1(x))))

Sharding: nodes dst-partitioned across 8 cores. Each sparse propagation:
  bf16 table of scaled features (AllGather'd into a Shared HBM scratch),
  bulk indexed gather (dma_gather, int16 idx, 32768-row windows, 4 SWDGE
  queues round-robin), segment-sum via one-hot fp8 [128 edge x 128 node]
  matmuls accumulated in PSUM per (window, block), accumulated across
  windows in an SBUF accumulator.

v2 (instruction-count oriented): host pre-transposes x so phase A needs no
PE transposes; gather indices resident in SBUF for all 4 passes; CH=32
gather chunks; per-block post-processing replaced by whole-width strided
DVE ops (AP broadcast_to); batched log_softmax; PSUM readouts split
across scalar (copy) and vector (add) engines.

All FP math on device; host does integer preprocessing only (CSR sort,
padding, one-hot construction, layout permutes).
"""
import os
import numpy as np
import ml_dtypes

import concourse.bass as bass
import concourse.bacc as bacc
import concourse.mybir as mybir
import concourse.tile as tile
from concourse import library_config
from concourse._compat import cdiv

F32 = mybir.dt.float32
BF16 = mybir.dt.bfloat16
FP8 = mybir.dt.float8e4
I16 = mybir.dt.int16
AF = mybir.ActivationFunctionType
OP = mybir.AluOpType

P = 128
NCORES = 8


class CFG:
    N = 100000
    F1 = 128          # input features
    H = 64            # hidden
    C = 40            # classes
    WIN = 32768       # gather window rows (int16 idx)
    CH = 8            # gather chunk size in tiles (1024 descs = SWDGE ring cap)
    XB = 14           # x blocks per phase-A chunk

    @property
    def OWN(self):
        return self.N // NCORES

    @property
    def NB(self):
        return cdiv(self.OWN, P)

    @property
    def PADN(self):
        return self.NB * P

    @property
    def GPAD(self):
        return NCORES * self.PADN

    @property
    def NWIN(self):
        return cdiv(self.GPAD, self.WIN)


cfg = CFG()

_cache = {}


# ---------------------------------------------------------------- host prep
def _row_of(v, c):
    """Permuted table row of global node v owned by core c (partition-major)."""
    l = v - c * cfg.OWN
    return c * cfg.PADN + (l % P) * cfg.NB + (l // P)


def preprocess(x, edge_index, W1, b1, W2, b2):
    src = np.asarray(edge_index[0], dtype=np.int64)
    dst = np.asarray(edge_index[1], dtype=np.int64)
    deg = np.bincount(dst, minlength=cfg.N).astype(np.float32)

    owner_s = src // cfg.OWN
    grow_s = _row_of(src, owner_s)          # permuted global table row of src

    per_core = []
    for c in range(NCORES):
        m = (dst >= c * cfg.OWN) & (dst < (c + 1) * cfg.OWN)
        es = grow_s[m]
        ld = dst[m] - c * cfg.OWN
        b = ld // P
        pcol = ld % P
        w = es // cfg.WIN
        order = np.lexsort((ld, b, w))
        per_core.append((w[order], b[order], es[order] % cfg.WIN, pcol[order]))

    # static per-(w,b) tile counts = max over cores
    t_wb = np.zeros((cfg.NWIN, cfg.NB), np.int64)
    for c in range(NCORES):
        w, b, _, _ = per_core[c]
        cnt = np.bincount(w * cfg.NB + b, minlength=cfg.NWIN * cfg.NB)
        t_wb = np.maximum(t_wb, cdiv_np(cnt.reshape(cfg.NWIN, cfg.NB), P))
    t_wb[0] = np.maximum(t_wb[0], 1)
    T = int(t_wb.sum())
    slots = T * P

    # group start slot offsets
    gstart = np.zeros(cfg.NWIN * cfg.NB + 1, np.int64)
    gstart[1:] = np.cumsum(t_wb.reshape(-1) * P)

    x = np.asarray(x, np.float32)
    core_inputs = []
    for c in range(NCORES):
        w, b, es, pcol = per_core[c]
        g = w * cfg.NB + b
        # slot of each edge: group base + rank within group
        changes = np.ones(len(g), bool)
        changes[1:] = g[1:] != g[:-1]
        grp_first = np.flatnonzero(changes)
        rank = np.arange(len(g)) - np.repeat(grp_first, np.diff(np.append(grp_first, len(g))))
        slot = gstart[g] + rank

        idx_full = np.zeros(slots, np.int64)           # window-local src row
        idx_full[slot] = es
        mcol = np.full(slots, -1, np.int64)            # one-hot col (-1 = pad)
        mcol[slot] = pcol

        # idx wrap: slot i -> [16g + i%16, i//16], int16
        iw = np.zeros((P, slots // 16), np.int16)
        a = idx_full.astype(np.int16)
        ii = np.arange(slots)
        for gg in range(8):
            iw[16 * gg + (ii % 16), ii // 16] = a

        # one-hot M [e, t, n] -> device layout [p=e, t, n] fp8
        mm = np.zeros((P, T, P), np.uint8)             # fp8e4m3 bits
        one = np.float32(1.0).astype(ml_dtypes.float8_e4m3).view(np.uint8)
        tt = ii // P
        ee = ii % P
        valid = mcol >= 0
        mm[ee[valid], tt[valid], mcol[valid]] = one
        mm = mm.view(ml_dtypes.float8_e4m3)

        # x slice transposed [f, b, p];  node l = b*P + p
        xs = np.zeros((cfg.F1, cfg.NB, P), np.float32)
        n = np.arange(cfg.OWN)
        xs[:, n // P, n % P] = x[c * cfg.OWN:(c + 1) * cfg.OWN].T
        dg = np.zeros((P, cfg.NB), np.float32)
        dg[n % P, n // P] = deg[c * cfg.OWN:(c + 1) * cfg.OWN]

        core_inputs.append(dict(
            x_in=np.ascontiguousarray(xs), deg_in=dg, idx_in=iw,
            m_in=np.ascontiguousarray(mm),
        ))

    W1 = np.asarray(W1, np.float32)
    W2 = np.asarray(W2, np.float32)
    w1cat = np.concatenate([W1[1], W1[2], W1[0]], axis=1)       # [F1, 3H]
    w2cat = np.concatenate([W2[1], W2[2], W2[0]], axis=1)       # [H, 3C]
    shared = dict(
        w1_in=np.ascontiguousarray(w1cat),
        w2_in=np.ascontiguousarray(w2cat),
        b1_in=np.ascontiguousarray(np.broadcast_to(np.asarray(b1, np.float32), (P, cfg.H)).copy()),
        b2_in=np.ascontiguousarray(np.broadcast_to(np.asarray(b2, np.float32), (P, cfg.C)).copy()),
        ident_in=np.eye(P, dtype=np.float32),
    )
    for ci in core_inputs:
        ci.update(shared)
    return core_inputs, t_wb


def cdiv_np(a, b):
    return -(-a // b)


# ---------------------------------------------------------------- device build
def build_bass(t_wb, max_phase=99, body_reps=1):
    NB, NWIN, F1, H, C = cfg.NB, cfg.NWIN, cfg.F1, cfg.H, cfg.C
    T = int(t_wb.sum())

    nc = bacc.Bacc("TRN2", target_bir_lowering=False, debug=False,
                   num_devices=NCORES, num_swdge_queues=4)

    x_in = nc.dram_tensor("x_in", [F1, NB, P], F32, kind="ExternalInput")
    deg_in = nc.dram_tensor("deg_in", [P, NB], F32, kind="ExternalInput")
    idx_in = nc.dram_tensor("idx_in", [P, T * P // 16], I16, kind="ExternalInput")
    m_in = nc.dram_tensor("m_in", [P, T, P], FP8, kind="ExternalInput")
    w1_in = nc.dram_tensor("w1_in", [F1, 3 * H], F32, kind="ExternalInput")
    w2_in = nc.dram_tensor("w2_in", [H, 3 * C], F32, kind="ExternalInput")
    b1_in = nc.dram_tensor("b1_in", [P, H], F32, kind="ExternalInput")
    b2_in = nc.dram_tensor("b2_in", [P, C], F32, kind="ExternalInput")
    ident_in = nc.dram_tensor("ident_in", [P, P], F32, kind="ExternalInput")
    out_d = nc.dram_tensor("out_d", [P, NB * C], F32, kind="ExternalOutput")

    ag_in = [nc.dram_tensor(f"ag{i}_in", [cfg.PADN, 2 * H], BF16, kind="Internal")
             for i in range(4)]
    tabs = [nc.dram_tensor(f"tab{i}", [cfg.GPAD, 2 * H], BF16, kind="Internal",
                           addr_space="Shared")
            for i in range(4)]

    CH = cfg.CH
    rg = [list(range(NCORES))]

    with tile.TileContext(nc) as tc:
        import contextlib
        ctx = contextlib.ExitStack()
        with ctx:
            cp = ctx.enter_context(tc.tile_pool(name="consts", bufs=1))
            bigp = ctx.enter_context(tc.tile_pool(name="big", bufs=1))
            xp = ctx.enter_context(tc.tile_pool(name="xin", bufs=2))
            gp = ctx.enter_context(tc.tile_pool(name="gather", bufs=3))
            mp = ctx.enter_context(tc.tile_pool(name="mtiles", bufs=3))
            fp = ctx.enter_context(tc.tile_pool(name="fin", bufs=4))
            pp = ctx.enter_context(tc.tile_pool(name="psum", bufs=6, space="PSUM"))

            nc.gpsimd.load_library(library_config.mlp)

            def scope(name):
                sid, _ = nc.enter_named_scope(name, False)
                return lambda: nc.leave_named_scope(name, sid, False)

            # ---------- constants ----------
            wcat1 = cp.tile([F1, 3 * H], F32)
            nc.sync.dma_start(out=wcat1[:], in_=w1_in[:])
            # cols [2H:3H] := W1[0] - W1[2]
            nc.vector.tensor_tensor(out=wcat1[:, 2 * H:3 * H], in0=wcat1[:, 2 * H:3 * H],
                                    in1=wcat1[:, H:2 * H], op=OP.subtract)
            wcat2 = cp.tile([H, 4 * C], F32)
            nc.sync.dma_start(out=wcat2[:, :3 * C], in_=w2_in[:])
            nc.vector.tensor_tensor(out=wcat2[:, 3 * C:], in0=wcat2[:, 2 * C:3 * C],
                                    in1=wcat2[:, C:2 * C], op=OP.subtract)
            b1r = cp.tile([P, H], F32)
            nc.sync.dma_start(out=b1r[:], in_=b1_in[:])
            b2r = cp.tile([P, C], F32)
            nc.sync.dma_start(out=b2r[:], in_=b2_in[:])
            ident = cp.tile([P, P], F32)
            nc.sync.dma_start(out=ident[:], in_=ident_in[:])

            degt = cp.tile([P, NB], F32)
            nc.sync.dma_start(out=degt[:], in_=deg_in[:])
            # dis = (deg>0) * sqrt(1/max(deg,1))
            dis = cp.tile([P, NB], F32)
            tmpc = cp.tile([P, NB], F32)
            nc.vector.tensor_scalar_max(tmpc[:], degt[:], 1.0)
            nc.vector.reciprocal(tmpc[:], tmpc[:])
            nc.scalar.activation(tmpc[:], tmpc[:], AF.Sqrt)
            mask = cp.tile([P, NB], F32)
            nc.vector.tensor_scalar(mask[:], degt[:], 0.0, None, op0=OP.is_gt)
            nc.vector.tensor_tensor(out=dis[:], in0=tmpc[:], in1=mask[:], op=OP.mult)
            negdis = cp.tile([P, NB], F32)
            nc.vector.tensor_scalar_mul(negdis[:], dis[:], -1.0)
            negdis2 = cp.tile([P, NB], F32)   # -dis^2
            nc.vector.tensor_tensor(out=negdis2[:], in0=negdis[:], in1=dis[:], op=OP.mult)
            neg2dis = cp.tile([P, NB], F32)   # -2 dis
            nc.vector.tensor_scalar_mul(neg2dis[:], dis[:], -2.0)

            def bP(v, inner):  # [P, NB] -> [P, NB, inner] stride-0 broadcast
                return v[:].unsqueeze(2).broadcast_to([P, NB, inner])

            def bF(v, inner):  # [P, inner] -> [P, NB, inner] stride-0 broadcast
                return v[:].unsqueeze(1).broadcast_to([P, NB, inner])

            # ---------- big SBUF state ----------
            # acc column scheme per block b: LOW = [:, b, 0:H], HIGH = [:, b, H:2H]
            acc = bigp.tile([P, NB, F1], F32)
            y0my2 = bigp.tile([P, NB, H], F32)   # later reused for h, then q2
            tabsb = bigp.tile([P, NB, 2 * H], BF16)
            out2 = bigp.tile([P, NB, C], F32)
            negmx = bigp.tile([P, NB], F32)
            smv = bigp.tile([P, NB], F32)
            # resident gather indices (same for all 4 passes)
            idxs = bigp.tile([P, T * P // 16], I16)
            nc.sync.dma_start(out=idxs[:], in_=idx_in[:])

            # group bookkeeping shared by all passes
            tmap = []
            for wv in range(NWIN):
                for b in range(NB):
                    n = int(t_wb[wv, b])
                    for k in range(n):
                        tmap.append((wv, b, k == 0, k == n - 1))

            qrr = [0]  # gather queue round-robin counter

            def run_pass(pi, wide):
                Fi = 2 * H if wide else H
                gt0 = 0
                ps_cur = [None]
                for w in range(NWIN):
                    tw = int(t_wb[w].sum())
                    wend = min((w + 1) * cfg.WIN, cfg.GPAD)
                    done = 0
                    while done < tw:
                        ch = min(CH, tw - done)
                        gb = gp.tile([P, CH, 2 * H], BF16, tag="gb")
                        nc.gpsimd.dma_gather(
                            gb[:, :ch, :], tabs[pi][w * cfg.WIN:wend, :],
                            idxs[:, gt0 * 8:(gt0 + ch) * 8], ch * P, ch * P,
                            2 * H,
                            queue_num=qrr[0] % int(os.environ.get("KERNEL_GQ", "4")))
                        qrr[0] += 1
                        mt = mp.tile([P, CH, P], FP8, tag="mt")
                        nc.sync.dma_start(out=mt[:, :ch, :], in_=m_in[:, gt0:gt0 + ch, :])
                        for k in range(ch):
                            gt = gt0 + k
                            w_, b_, first, last = tmap[gt]
                            if first:
                                ps_cur[0] = pp.tile([P, Fi], F32, tag="ps", name="ps_prop")
                            nc.tensor.matmul(ps_cur[0][:], lhsT=mt[:, k, :],
                                             rhs=gb[:, k, :Fi], start=first, stop=last)
                            if last:
                                if wide:
                                    dstsl = acc[:, b_, :]
                                else:
                                    # pass2 -> HIGH, pass3 -> LOW, pass4 -> HIGH
                                    off = 0 if pi == 2 else H
                                    dstsl = acc[:, b_, off:off + H]
                                if w_ == 0:
                                    nc.scalar.activation(dstsl, ps_cur[0][:], AF.Copy)
                                else:
                                    nc.vector.tensor_tensor(out=dstsl, in0=dstsl,
                                                            in1=ps_cur[0][:], op=OP.add)
                        gt0 += ch
                        done += ch

            def logits_pass(getrow, wsel, first_write):
                """out2 (+)= (getrow(b) [P,H]) @ wcat2[:, wsel]  via PE transposes."""
                for b in range(NB):
                    ht_ps = pp.tile([P, P], F32, tag="ps")
                    nc.tensor.transpose(out=ht_ps[:H, :], in_=getrow(b), identity=ident[:])
                    ht = fp.tile([H, P], F32, tag="ht")
                    nc.vector.tensor_copy(out=ht[:], in_=ht_ps[:H, :])
                    lg = pp.tile([P, C], F32, tag="ps")
                    nc.tensor.matmul(lg[:], lhsT=ht[:], rhs=wcat2[:, wsel], start=True, stop=True)
                    if first_write:
                        nc.scalar.activation(out2[:, b, :], lg[:], AF.Copy)
                    else:
                        nc.vector.tensor_tensor(out=out2[:, b, :], in0=out2[:, b, :],
                                                in1=lg[:], op=OP.add)

            for _rep in range(body_reps):
                # ---------- phase A: y = x @ W1cat; write ytilde table ----------
                if max_phase >= 1:
                    _s = scope("densA")
                    for c0 in range(0, NB, cfg.XB):
                        nb = min(cfg.XB, NB - c0)
                        xs = xp.tile([F1, cfg.XB, P], F32, tag="xs")
                        nc.sync.dma_start(out=xs[:, :nb, :], in_=x_in[:, c0:c0 + nb, :])
                        for j in range(nb):
                            b = c0 + j
                            y_ps = pp.tile([P, 3 * H], F32, tag="ps")
                            nc.tensor.matmul(y_ps[:], lhsT=xs[:, j, :], rhs=wcat1[:],
                                             start=True, stop=True)
                            nc.vector.tensor_scalar(tabsb[:, b, :], y_ps[:, :2 * H],
                                                    dis[:, b:b + 1], None, op0=OP.mult)
                            nc.scalar.activation(y0my2[:, b, :], y_ps[:, 2 * H:3 * H], AF.Copy)
                    _s()
                    _s = scope("ag0")
                    nc.sync.dma_start(out=ag_in[0][:], in_=tabsb[:])
                    nc.gpsimd.collective_compute(
                        "AllGather", OP.bypass, replica_groups=rg,
                        ins=[ag_in[0][:].opt()], outs=[tabs[0][:].opt()])
                    _s()

                # ===== pass 1 (wide): acc = [u1 | u2] =====
                if max_phase >= 2:
                    _s = scope("prop1")
                    for _ in range(int(os.environ.get("KERNEL_REPS", "1"))):
                        run_pass(0, wide=True)
                    _s()
                    _s = scope("post1")
                    # tab2 = -dis^2 * u2 ; z1 = -dis * u1 (in place over u1)
                    nc.vector.tensor_tensor(out=tabsb[:, :, :H], in0=acc[:, :, H:],
                                            in1=bP(negdis2, H), op=OP.mult)
                    nc.vector.tensor_tensor(out=acc[:, :, :H], in0=acc[:, :, :H],
                                            in1=bP(negdis, H), op=OP.mult)
                    _s()
                    _s = scope("ag1")
                    nc.sync.dma_start(out=ag_in[1][:], in_=tabsb[:])
                    nc.gpsimd.collective_compute(
                        "AllGather", OP.bypass, replica_groups=rg,
                        ins=[ag_in[1][:].opt()], outs=[tabs[1][:].opt()])
                    _s()

                # ===== pass 2: acc.HIGH = u = A^T ztilde2 ; h = relu(...) =====
                if max_phase >= 3:
                    _s = scope("prop2")
                    run_pass(1, wide=False)
                    _s()
                    _s = scope("post2")
                    # h = relu(y0my2 + z1 + -2dis*u + b1)  -> into y0my2
                    hv = y0my2
                    nc.vector.tensor_tensor(out=acc[:, :, H:], in0=acc[:, :, H:],
                                            in1=bP(neg2dis, H), op=OP.mult)
                    nc.vector.tensor_tensor(out=hv[:, :, :], in0=hv[:, :, :],
                                            in1=acc[:, :, H:], op=OP.add)
                    nc.vector.tensor_tensor(out=hv[:, :, :], in0=hv[:, :, :],
                                            in1=acc[:, :, :H], op=OP.add)
                    nc.vector.tensor_tensor(out=hv[:, :, :], in0=hv[:, :, :],
                                            in1=bF(b1r, H), op=OP.add)
                    nc.vector.tensor_scalar_max(hv[:, :, :], hv[:, :, :], 0.0)
                    nc.vector.tensor_tensor(out=tabsb[:, :, :H], in0=hv[:, :, :],
                                            in1=bP(dis, H), op=OP.mult)
                    # out2 = h @ (W2[0]-W2[2])
                    logits_pass(lambda b: hv[:, b, :], slice(3 * C, 4 * C),
                                first_write=True)
                    _s()
                    _s = scope("ag2")
                    nc.sync.dma_start(out=ag_in[2][:], in_=tabsb[:])
                    nc.gpsimd.collective_compute(
                        "AllGather", OP.bypass, replica_groups=rg,
                        ins=[ag_in[2][:].opt()], outs=[tabs[2][:].opt()])
                    _s()

                # ===== pass 3: acc.LOW = u = A^T htilde ; q1 = P(h) =====
                if max_phase >= 4:
                    _s = scope("prop3")
                    run_pass(2, wide=False)
                    _s()
                    _s = scope("post3")
                    nc.vector.tensor_tensor(out=tabsb[:, :, :H], in0=acc[:, :, :H],
                                            in1=bP(negdis2, H), op=OP.mult)
                    nc.vector.tensor_tensor(out=acc[:, :, :H], in0=acc[:, :, :H],
                                            in1=bP(negdis, H), op=OP.mult)
                    # out2 += q1 @ W2[1]
                    logits_pass(lambda b: acc[:, b, :H], slice(0, C), first_write=False)
                    _s()
                    _s = scope("ag3")
                    nc.sync.dma_start(out=ag_in[3][:], in_=tabsb[:])
                    nc.gpsimd.collective_compute(
                        "AllGather", OP.bypass, replica_groups=rg,
                        ins=[ag_in[3][:].opt()], outs=[tabs[3][:].opt()])
                    _s()

                # ===== pass 4: acc.HIGH = u = A^T q1tilde ; logits + softmax =====
                if max_phase >= 5:
                    _s = scope("prop4")
                    run_pass(3, wide=False)
                    _s()
                    _s = scope("post4")
                    # q2 = -2dis*u (into y0my2; acc.LOW becomes Exp scratch)
                    nc.vector.tensor_tensor(out=y0my2[:, :, :], in0=acc[:, :, H:],
                                            in1=bP(neg2dis, H), op=OP.mult)
                    logits_pass(lambda b: y0my2[:, b, :], slice(C, 2 * C),
                                first_write=False)
                    # batched log_softmax over C
                    nc.vector.tensor_tensor(out=out2[:, :, :], in0=out2[:, :, :],
                                            in1=bF(b2r, C), op=OP.add)
                    nc.vector.tensor_reduce(out=negmx[:].unsqueeze(2), in_=out2[:, :, :],
                                            op=OP.max, axis=mybir.AxisListType.X,
                                            negate=True)
                    nc.vector.tensor_tensor(out=out2[:, :, :], in0=out2[:, :, :],
                                            in1=bP(negmx, C), op=OP.add)
                    ex = acc[:, :, :C]  # scratch
                    nc.scalar.activation(ex, out2[:, :, :], AF.Exp)
                    nc.vector.tensor_reduce(out=smv[:].unsqueeze(2), in_=ex,
                                            op=OP.add, axis=mybir.AxisListType.X)
                    nc.scalar.activation(smv[:], smv[:], AF.Ln)
                    nc.vector.tensor_tensor(out=out2[:, :, :], in0=out2[:, :, :],
                                            in1=bP(smv, C), op=OP.subtract)
                    nc.sync.dma_start(out=out_d[:], in_=out2[:, :, :])
                    _s()

    nc.compile()
    return nc


# ---------------------------------------------------------------- entry point
def kernel(**inputs):
    core_inputs, t_wb = preprocess(
        inputs["x"], inputs["edge_index"], inputs["W1"], inputs["b1"],
        inputs["W2"], inputs["b2"])

    key = (cfg.N, tuple(t_wb.reshape(-1).tolist()))
    if key not in _cache:
        _cache[key] = build_bass(t_wb, int(os.environ.get("KERNEL_PHASES", "99")))
    nc = _cache[key]

    if os.environ.get("KERNEL_SIM"):
        from concourse.bass_interp import MultiCoreSim
        sim = MultiCoreSim(nc, NCORES)
        for c in range(NCORES):
            for k, v in core_inputs[c].items():
                sim.cores[c].tensor(k)[:] = v
        sim.simulate()
        results = [{"out_d": np.asarray(sim.cores[c].mem_tensor("out_d"))}
                   for c in range(NCORES)]
    else:
        from concourse.bass_utils import run_bass_kernel_spmd
        r = run_bass_kernel_spmd(nc, core_inputs, core_ids=list(range(NCORES)))
        results = r.results

    out = np.empty((cfg.N, cfg.C), np.float32)
    n = np.arange(cfg.OWN)
    for c in range(NCORES):
        od = results[c]["out_d"].reshape(P, cfg.NB, cfg.C)
        out[c * cfg.OWN:(c + 1) * cfg.OWN] = od[n % P, n // P, :]
    return out
